# revision 22
# baseline (speedup 1.0000x reference)
"""MinkowskiInstanceNorm (segment-reduce instance norm) on 8 Trainium2 cores.

Strategy: seg_ids are sorted, so each segment is a contiguous run of rows.
With num_segments == n_cores == 8, core j owns segment j outright: it
computes sum(x) and sum(x^2) over its rows (padded with zeros so one SPMD
program serves all cores), derives mean / inv_std / affine on-device, and
normalizes in a second pass.  No cross-core communication is needed; the
host only slices rows per segment and stitches the outputs back in order.

Perf design (HBM-bound problem; ~430 GB/s/core achievable):
  - The device slab is fp16 (host casts on pack, upcasts on unpack),
    halving HBM traffic vs fp32.  Quantization error ~5e-4 relative,
    far inside the correctness gate.
  - The whole per-core slab stays RESIDENT in SBUF across both passes,
    so HBM traffic is exactly one read + one write of the data.
  - Layout: channels on partitions — partition p = rb*32 + c (rb =
    row-block 0..3, c = channel), free axis = column f holding row
    rb*TOTF + f of that block; one flat [128, TOTF] slab, split into
    tiles of DESCENDING width so the last tile's compute tail after the
    final load is tiny.
  - Pass-1 per tile: PE folds sum(x) via 0/1-selector matmuls
    accumulated in a per-tile PSUM region (fp16 rhs streams ~3 cols/
    cycle, and PE is otherwise idle); sum(x^2) is split DVE
    (scalar_tensor_tensor, 1x ~1.07 ns/elem) / ACT (Square+accum,
    ~0.886 ns/elem); ACT also drains each tile's PSUM region as soon as
    that tile's matmuls finish, so nothing but the last tile's work
    lands in the stats gap.
  - Pass 2 normalizes smallest-tile-first with one 4x-mode DVE
    tensor_scalar per tile and alternates stores across both HWDGE
    rings (sync + ACT) for sustained store bandwidth.
"""

from contextlib import ExitStack

import numpy as np

C = 32  # channels
P = 128  # SBUF partitions
RB = P // C  # row blocks (4)
NCORES = 8
EPS = 1e-8
T_TILES = 8
# descending tile-width fractions (sum = 1); last tile ~6% so the
# compute tail after the final load is short
FRACS = [0.15, 0.15, 0.145, 0.14, 0.135, 0.125, 0.105, 0.05]
DVE_SHARE = 0.48  # leading share of each tile's sum(x^2) on DVE
# fully-resident limit: slab(2*TOTF) + obuf(2 slots) + scratch must fit
TOTF_RESIDENT_MAX = 65536
FD_STREAM = 8192  # uniform tile width for the streamed fallback

_PROGRAMS = {}
LAST_RESULTS = None  # BassKernelResults of the most recent SPMD launch


def _cfg(maxc):
    """Pick (totf, fds, resident) covering maxc rows (= RB*totf capacity)."""
    need = max(int(maxc), 1)
    totf = -(-need // (RB * 64)) * 64
    if totf <= TOTF_RESIDENT_MAX:
        fds = []
        off = 0
        for i, fr in enumerate(FRACS):
            if i == len(FRACS) - 1:
                fd = totf - off
            else:
                fd = max((int(totf * fr) // 64) * 64, 64)
            fds.append(fd)
            off += fd
        assert sum(fds) == totf and all(f > 0 for f in fds)
        return totf, fds, True
    t = -(-need // (RB * FD_STREAM))
    return t * FD_STREAM, [FD_STREAM] * t, False


def _emit(nc, tc, ctx, tensors, totf, fds, resident):
    from concourse import mybir

    dt = mybir.dt
    AX = mybir.AxisListType
    OP = mybir.AluOpType
    AF = mybir.ActivationFunctionType

    x_d, invn_d, w_d, b_d, s128_d, s16_d, s32_d, o_d = tensors
    xv = x_d.ap()  # [P, TOTF]
    ov = o_d.ap()
    T = len(fds)
    offs = [sum(fds[:i]) for i in range(T)]
    fdmax = max(fds)

    const = ctx.enter_context(tc.tile_pool(name="const", bufs=1))
    xpool = ctx.enter_context(tc.tile_pool(name="xpool", bufs=3))
    ypool = ctx.enter_context(tc.tile_pool(name="ypool", bufs=2))
    psum = ctx.enter_context(tc.tile_pool(name="psum", bufs=1, space="PSUM"))

    if resident:
        RES = T
        res = const.tile([P, totf], dt.float16, name="res")
    else:
        budget = 190 * 1024
        fixed = 2 * 2 * fdmax + 3 * fdmax + 2 * fdmax + 4096
        RES = min(T, max((budget - fixed - 10 * fdmax) // (fdmax * 2), 0))
        res = (
            const.tile([P, sum(fds[:RES])], dt.float16, name="res")
            if RES
            else None
        )

    # output ring buffer: 3 slots of the largest tile width
    obuf = const.tile([P, 3 * fdmax], dt.float16)
    dvemax = (int(fdmax * DVE_SHARE) // 32) * 32
    ttrscr = const.tile([P, dvemax], dt.float16)
    sqscr = const.tile([P, fdmax - dvemax + 32], dt.float16)
    drscr = const.tile([C, 1024], dt.float32)
    drscr2 = None if resident else const.tile([C, 4096], dt.float32)

    qparts = const.tile([P, 2 * T], dt.float32)
    st_s = const.tile([C, 4 if resident else -(-T // 8)], dt.float32)

    # Tile loads first so the sync-ring FIFO streams them back-to-back;
    # consts ride the gpsimd (SWDGE) ring so sel16 beats tile 0's matmul.
    for i in range(RES):
        nc.sync.dma_start(
            out=res[:, offs[i] : offs[i] + fds[i]],
            in_=xv[:, offs[i] : offs[i] + fds[i]],
        )

    sel16 = const.tile([P, C], dt.float16)
    nc.gpsimd.dma_start(out=sel16[:], in_=s16_d.ap())
    invn = const.tile([C, 1], dt.float32)
    nc.gpsimd.dma_start(out=invn[:], in_=invn_d.ap())
    wt = const.tile([C, 1], dt.float32)
    nc.gpsimd.dma_start(out=wt[:], in_=w_d.ap())
    bt = const.tile([C, 1], dt.float32)
    nc.gpsimd.dma_start(out=bt[:], in_=b_d.ap())
    sel128 = const.tile([P, C], dt.float32)
    nc.gpsimd.dma_start(out=sel128[:], in_=s128_d.ap())
    sel32 = const.tile([C, P], dt.float32)
    nc.gpsimd.dma_start(out=sel32[:], in_=s32_d.ap())

    # warm the ACT Sqrt table set now so the stats chain doesn't pay the
    # table load later
    epsv = const.tile([C, 1], dt.float32)
    nc.vector.memset(epsv[:], EPS)
    warm = const.tile([C, 1], dt.float32)
    nc.scalar.activation(warm[:], epsv[:], AF.Sqrt, bias=epsv[:])

    # one 512-col PSUM region per tile (slice r=i%8 of one big tile);
    # every chunk matmul writes cols [0:512) of its region (the final
    # partial chunk overlaps cols [0:rem)), so regions are fully valid
    # and group drains can span several regions at once.
    nreg = min(T, 8)
    bigps = psum.tile([P, nreg * 512], dt.float32)

    xts = []
    for i in range(T):
        if i < RES:
            xt = res[:, offs[i] : offs[i] + fds[i]]
        else:
            xt = xpool.tile([P, fds[i]], dt.float16, tag="xt")
            nc.sync.dma_start(out=xt[:], in_=xv[:, offs[i] : offs[i] + fds[i]])
        xts.append(xt)
        fd = fds[i]
        reg = bigps[:, (i % nreg) * 512 : (i % nreg + 1) * 512]
        # sum(x): chunk matmuls against the 0/1 selector, accumulated in
        # this tile's PSUM region (overlapping columns just add up)
        pos = 0
        first = True
        while pos < fd:
            w_cols = min(512, fd - pos)
            nc.tensor.matmul(
                reg[:C, :w_cols],
                lhsT=sel16[:],
                rhs=xt[:, pos : pos + w_cols],
                start=first,
                stop=(pos + w_cols >= fd),
            )
            first = False
            pos += w_cols
        # sum(x^2): leading DVE_F on DVE, trailing on ACT
        dve_f = (int(fd * DVE_SHARE) // 32) * 32
        nc.vector.scalar_tensor_tensor(
            out=ttrscr[:, :dve_f],
            in0=xt[:, :dve_f],
            scalar=1.0,
            in1=xt[:, :dve_f],
            op0=OP.mult,
            op1=OP.mult,
            accum_out=qparts[:, i : i + 1],
        )
        nc.scalar.activation(
            sqscr[:, : fd - dve_f],
            xt[:, dve_f:],
            AF.Square,
            accum_out=qparts[:, T + i : T + i + 1],
        )
        # group drains: regions 0..5 leave PSUM on ACT while later tiles
        # still stream (2 ops), regions 6..7 go to DVE in the tail
        if resident and i == 5:
            for g in range(3):
                nc.scalar.activation(
                    drscr[:],
                    bigps[:C, g * 1024 : (g + 1) * 1024],
                    AF.Copy,
                    accum_out=st_s[:, g : g + 1],
                )
        elif not resident and (i % nreg == nreg - 1 or i == T - 1):
            lo = (i - i % nreg) % 1  # group start region is always 0
            nreg_used = i % nreg + 1
            nc.scalar.activation(
                drscr2[:, : nreg_used * 512],
                bigps[:C, : nreg_used * 512],
                AF.Copy,
                accum_out=st_s[:, i // nreg : i // nreg + 1],
            )

    if resident:
        nc.vector.tensor_reduce(
            out=st_s[:, 3:4], in_=bigps[:C, 3072:4096], axis=AX.X, op=OP.add
        )
    ssum = const.tile([C, 1], dt.float32)
    nc.vector.tensor_reduce(out=ssum[:], in_=st_s[:], axis=AX.X, op=OP.add)
    qred = const.tile([P, 1], dt.float32)
    nc.vector.tensor_reduce(out=qred[:], in_=qparts[:], axis=AX.X, op=OP.add)
    # fold the RB row-blocks of each channel: [32, 1] = sel128.T @ qred
    nc.tensor.matmul(
        bigps[:C, 0:1], lhsT=sel128[:], rhs=qred[:], start=True, stop=True
    )

    mean = const.tile([C, 1], dt.float32)
    nc.vector.tensor_scalar_mul(mean[:], ssum[:], invn[:])
    ex2 = const.tile([C, 1], dt.float32)
    nc.vector.tensor_scalar_mul(ex2[:], bigps[:C, 0:1], invn[:])
    msq = const.tile([C, 1], dt.float32)
    nc.vector.tensor_mul(msq[:], mean[:], mean[:])
    var = const.tile([C, 1], dt.float32)
    nc.vector.tensor_sub(var[:], ex2[:], msq[:])
    std = const.tile([C, 1], dt.float32)
    nc.scalar.activation(std[:], var[:], AF.Sqrt, bias=epsv[:])
    istd = const.tile([C, 1], dt.float32)
    nc.vector.reciprocal(istd[:], std[:])
    # ab = [A | B]: A = w/std, B = b - mean*A
    ab = const.tile([C, 2], dt.float32)
    nc.vector.tensor_mul(ab[:, 0:1], istd[:], wt[:])
    nc.vector.tensor_mul(ab[:, 1:2], mean[:], ab[:, 0:1])
    nc.vector.tensor_sub(ab[:, 1:2], bt[:], ab[:, 1:2])
    # broadcast A/B back to all 128 partitions: [128, 2] = sel32.T @ ab
    nc.tensor.matmul(
        bigps[:, 512:514], lhsT=sel32[:], rhs=ab[:], start=True, stop=True
    )
    ab128 = const.tile([P, 2], dt.float32)
    nc.scalar.copy(ab128[:], bigps[:, 512:514])

    # Pass 2: smallest tile first (fast first store), then descending so
    # the final store is small too.
    if resident:
        by_size = sorted(range(T), key=lambda i: -fds[i])
        order = [by_size[-1]] + by_size[:-1]
    else:
        order = list(range(T))
    for n, i in enumerate(order):
        fd = fds[i]
        if i < RES:
            yt = xts[i]
        else:
            yt = ypool.tile([P, fd], dt.float16, tag="yt")
            nc.sync.dma_start(out=yt[:], in_=xv[:, offs[i] : offs[i] + fd])
        slot = (n % 3) * fdmax
        ot = obuf[:, slot : slot + fd]
        nc.vector.tensor_scalar(
            out=ot[:],
            in0=yt[:],
            scalar1=ab128[:, 0:1],
            scalar2=ab128[:, 1:2],
            op0=OP.mult,
            op1=OP.add,
        )
        # alternate stores across both HWDGE rings (sync is idle in pass 2)
        if n % 2 == 0:
            nc.scalar.dma_start(out=ov[:, offs[i] : offs[i] + fd], in_=ot[:])
        else:
            nc.sync.dma_start(out=ov[:, offs[i] : offs[i] + fd], in_=ot[:])


def _get_program(totf, fds, resident):
    key = (totf, tuple(fds), resident)
    if key in _PROGRAMS:
        return _PROGRAMS[key]
    import concourse.tile as tile
    from concourse import bacc, mybir

    dt = mybir.dt
    nc = bacc.Bacc(
        "TRN2",
        target_bir_lowering=False,
        debug=False,
        enable_asserts=False,
        num_devices=NCORES,
    )
    x_d = nc.dram_tensor("x", [P, totf], dt.float16, kind="ExternalInput")
    invn_d = nc.dram_tensor("invn", [C, 1], dt.float32, kind="ExternalInput")
    w_d = nc.dram_tensor("w", [C, 1], dt.float32, kind="ExternalInput")
    b_d = nc.dram_tensor("b", [C, 1], dt.float32, kind="ExternalInput")
    s128_d = nc.dram_tensor("sel128", [P, C], dt.float32, kind="ExternalInput")
    s16_d = nc.dram_tensor("sel16", [P, C], dt.float16, kind="ExternalInput")
    s32_d = nc.dram_tensor("sel32", [C, P], dt.float32, kind="ExternalInput")
    o_d = nc.dram_tensor("o", [P, totf], dt.float16, kind="ExternalOutput")

    tensors = (x_d, invn_d, w_d, b_d, s128_d, s16_d, s32_d, o_d)
    with tile.TileContext(nc) as tc:
        with ExitStack() as ctx:
            _emit(nc, tc, ctx, tensors, totf, fds, resident)

    nc.finalize()
    _PROGRAMS[key] = nc
    return nc


def _pack(rows, totf):
    """rows [n, C] f32 -> [128, TOTF] f16: partition rb*32+c, column f
    holds row rb*TOTF + f of channel c; zero padded."""
    xp = np.zeros((RB * totf, C), dtype=np.float16)
    xp[: rows.shape[0]] = rows
    return np.ascontiguousarray(
        xp.reshape(RB, totf, C).transpose(0, 2, 1).reshape(P, totf)
    )


def _unpack(slab, n):
    """[128, TOTF] f16 -> rows [n, C] f32."""
    totf = slab.shape[1]
    return (
        slab.reshape(RB, C, totf)
        .transpose(0, 2, 1)
        .reshape(RB * totf, C)[:n]
        .astype(np.float32)
    )


def kernel(feats, seg_ids, weight, bias, num_segments, **_):
    from concourse.bass_utils import run_bass_kernel_spmd

    feats = np.ascontiguousarray(np.asarray(feats), dtype=np.float32)
    seg = np.asarray(seg_ids)
    w = np.asarray(weight, dtype=np.float32).reshape(C, 1)
    b = np.asarray(bias, dtype=np.float32).reshape(C, 1)
    S = int(num_segments)
    N = feats.shape[0]

    assert (np.diff(seg) >= 0).all(), "seg_ids must be sorted"
    bounds = np.searchsorted(seg, np.arange(S + 1)).astype(np.int64)
    counts = np.diff(bounds)

    sel128 = np.ascontiguousarray(np.tile(np.eye(C, dtype=np.float32), (RB, 1)))
    sel32 = np.ascontiguousarray(sel128.T)
    sel16 = sel128.astype(np.float16)

    out = np.empty((N, C), dtype=np.float32)
    for g0 in range(0, S, NCORES):
        gsegs = list(range(g0, min(g0 + NCORES, S)))
        maxc = max(int(counts[s]) for s in gsegs)
        totf, fds, resident = _cfg(maxc)
        nc = _get_program(totf, fds, resident)
        in_maps = []
        for j in range(NCORES):
            n_j = 1
            if j < len(gsegs):
                s = gsegs[j]
                n_j = max(int(counts[s]), 1)
                rows = feats[bounds[s] : bounds[s + 1]]
            else:
                rows = np.zeros((0, C), dtype=np.float32)
            in_maps.append(
                {
                    "x": _pack(rows, totf),
                    "invn": np.full((C, 1), 1.0 / n_j, dtype=np.float32),
                    "w": w,
                    "b": b,
                    "sel128": sel128,
                    "sel16": sel16,
                    "sel32": sel32,
                }
            )
        r = run_bass_kernel_spmd(nc, in_maps, list(range(NCORES)))
        global LAST_RESULTS
        LAST_RESULTS = r
        results = r.results
        for j, s in enumerate(gsegs):
            out[bounds[s] : bounds[s + 1]] = _unpack(results[j]["o"], int(counts[s]))
    return out


# revision 24
# speedup vs baseline: 1.0666x; 1.0666x over previous
"""MinkowskiInstanceNorm (segment-reduce instance norm) on 8 Trainium2 cores.

Strategy: seg_ids are sorted, so each segment is a contiguous run of rows.
With num_segments == n_cores == 8, core j owns segment j outright: it
computes sum(x) and sum(x^2) over its rows (padded with zeros so one SPMD
program serves all cores), derives mean / inv_std / affine on-device, and
normalizes in a second pass.  No cross-core communication is needed; the
host only slices rows per segment and stitches the outputs back in order.

Perf design (HBM-bound problem; ~430 GB/s/core achievable):
  - The device slab is fp16 (host casts on pack, upcasts on unpack),
    halving HBM traffic vs fp32.  Quantization error ~5e-4 relative,
    far inside the correctness gate.
  - The whole per-core slab stays RESIDENT in SBUF across both passes,
    so HBM traffic is exactly one read + one write of the data.
  - Layout: channels on partitions — partition p = rb*32 + c (rb =
    row-block 0..3, c = channel), free axis = column f holding row
    rb*TOTF + f of that block; one flat [128, TOTF] slab, split into
    tiles of DESCENDING width so the last tile's compute tail after the
    final load is tiny.
  - Pass-1 per tile: PE folds sum(x) via 0/1-selector matmuls
    accumulated in a per-tile PSUM region (fp16 rhs streams ~3 cols/
    cycle, and PE is otherwise idle); sum(x^2) is split DVE
    (scalar_tensor_tensor, 1x ~1.07 ns/elem) / ACT (Square+accum,
    ~0.886 ns/elem); ACT also drains each tile's PSUM region as soon as
    that tile's matmuls finish, so nothing but the last tile's work
    lands in the stats gap.
  - Pass 2 normalizes smallest-tile-first with one 4x-mode DVE
    tensor_scalar per tile and alternates stores across both HWDGE
    rings (sync + ACT) for sustained store bandwidth.
"""

from contextlib import ExitStack

import numpy as np

C = 32  # channels
P = 128  # SBUF partitions
RB = P // C  # row blocks (4)
NCORES = 8
EPS = 1e-8
T_TILES = 8
# descending tile-width fractions (sum = 1); last tile ~6% so the
# compute tail after the final load is short
FRACS = [0.15, 0.15, 0.14, 0.14, 0.13, 0.12, 0.10, 0.07]
DVE_SHARE = 0.48  # leading share of each tile's sum(x^2) on DVE
# fully-resident limit: slab(2*TOTF) + obuf(2 slots) + scratch must fit
TOTF_RESIDENT_MAX = 65536
FD_STREAM = 8192  # uniform tile width for the streamed fallback

_PROGRAMS = {}
LAST_RESULTS = None  # BassKernelResults of the most recent SPMD launch


def _cfg(maxc):
    """Pick (totf, fds, resident) covering maxc rows (= RB*totf capacity)."""
    need = max(int(maxc), 1)
    totf = -(-need // (RB * 64)) * 64
    if totf <= TOTF_RESIDENT_MAX:
        fds = []
        off = 0
        for i, fr in enumerate(FRACS):
            if i == len(FRACS) - 1:
                fd = totf - off
            else:
                fd = max((int(totf * fr) // 64) * 64, 64)
            fds.append(fd)
            off += fd
        assert sum(fds) == totf and all(f > 0 for f in fds)
        return totf, fds, True
    t = -(-need // (RB * FD_STREAM))
    return t * FD_STREAM, [FD_STREAM] * t, False


def _emit(nc, tc, ctx, tensors, totf, fds, resident):
    from concourse import mybir

    dt = mybir.dt
    AX = mybir.AxisListType
    OP = mybir.AluOpType
    AF = mybir.ActivationFunctionType

    x_d, invn_d, w_d, b_d, s128_d, s16_d, s32_d, o_d = tensors
    xv = x_d.ap()  # [P, TOTF]
    ov = o_d.ap()
    T = len(fds)
    offs = [sum(fds[:i]) for i in range(T)]
    fdmax = max(fds)

    const = ctx.enter_context(tc.tile_pool(name="const", bufs=1))
    xpool = ctx.enter_context(tc.tile_pool(name="xpool", bufs=3))
    ypool = ctx.enter_context(tc.tile_pool(name="ypool", bufs=2))
    psum = ctx.enter_context(tc.tile_pool(name="psum", bufs=1, space="PSUM"))

    if resident:
        RES = T
        res = const.tile([P, totf], dt.float16, name="res")
    else:
        budget = 190 * 1024
        fixed = 2 * 2 * fdmax + 3 * fdmax + 2 * fdmax + 4096
        RES = min(T, max((budget - fixed - 10 * fdmax) // (fdmax * 2), 0))
        res = (
            const.tile([P, sum(fds[:RES])], dt.float16, name="res")
            if RES
            else None
        )

    # output ring buffer: 3 slots of the largest tile width
    obuf = const.tile([P, 3 * fdmax], dt.float16)
    dvemax = (int(fdmax * DVE_SHARE) // 32) * 32
    ttrscr = const.tile([P, dvemax], dt.float16)
    sqscr = const.tile([P, fdmax - dvemax + 32], dt.float16)
    drscr = const.tile([C, 1024], dt.float32)
    drscr2 = None if resident else const.tile([C, 4096], dt.float32)

    qparts = const.tile([P, 2 * T], dt.float32)
    st_s = const.tile([C, 4 if resident else -(-T // 8)], dt.float32)

    # Tile loads first so the sync-ring FIFO streams them back-to-back;
    # consts ride the gpsimd (SWDGE) ring so sel16 beats tile 0's matmul.
    for i in range(RES):
        nc.sync.dma_start(
            out=res[:, offs[i] : offs[i] + fds[i]],
            in_=xv[:, offs[i] : offs[i] + fds[i]],
        )

    sel16 = const.tile([P, C], dt.float16)
    nc.gpsimd.dma_start(out=sel16[:], in_=s16_d.ap())
    invn = const.tile([C, 1], dt.float32)
    nc.gpsimd.dma_start(out=invn[:], in_=invn_d.ap())
    wt = const.tile([C, 1], dt.float32)
    nc.gpsimd.dma_start(out=wt[:], in_=w_d.ap())
    bt = const.tile([C, 1], dt.float32)
    nc.gpsimd.dma_start(out=bt[:], in_=b_d.ap())
    sel128 = const.tile([P, C], dt.float32)
    nc.gpsimd.dma_start(out=sel128[:], in_=s128_d.ap())
    sel32 = const.tile([C, P], dt.float32)
    nc.gpsimd.dma_start(out=sel32[:], in_=s32_d.ap())

    # warm the ACT Sqrt table set now so the stats chain doesn't pay the
    # table load later
    epsv = const.tile([C, 1], dt.float32)
    nc.vector.memset(epsv[:], EPS)
    warm = const.tile([C, 1], dt.float32)
    nc.scalar.activation(warm[:], epsv[:], AF.Sqrt, bias=epsv[:])

    # one 512-col PSUM region per tile (slice r=i%8 of one big tile);
    # every chunk matmul writes cols [0:512) of its region (the final
    # partial chunk overlaps cols [0:rem)), so regions are fully valid
    # and group drains can span several regions at once.
    nreg = min(T, 8)
    bigps = psum.tile([P, nreg * 512], dt.float32)

    xts = []
    for i in range(T):
        if i < RES:
            xt = res[:, offs[i] : offs[i] + fds[i]]
        else:
            xt = xpool.tile([P, fds[i]], dt.float16, tag="xt")
            nc.sync.dma_start(out=xt[:], in_=xv[:, offs[i] : offs[i] + fds[i]])
        xts.append(xt)
        fd = fds[i]
        reg = bigps[:, (i % nreg) * 512 : (i % nreg + 1) * 512]
        # sum(x): chunk matmuls against the 0/1 selector, accumulated in
        # this tile's PSUM region (overlapping columns just add up)
        pos = 0
        first = True
        while pos < fd:
            w_cols = min(512, fd - pos)
            nc.tensor.matmul(
                reg[:C, :w_cols],
                lhsT=sel16[:],
                rhs=xt[:, pos : pos + w_cols],
                start=first,
                stop=(pos + w_cols >= fd),
            )
            first = False
            pos += w_cols
        # sum(x^2): leading DVE_F on DVE, trailing on ACT
        dve_f = (int(fd * DVE_SHARE) // 32) * 32
        nc.vector.scalar_tensor_tensor(
            out=ttrscr[:, :dve_f],
            in0=xt[:, :dve_f],
            scalar=1.0,
            in1=xt[:, :dve_f],
            op0=OP.mult,
            op1=OP.mult,
            accum_out=qparts[:, i : i + 1],
        )
        nc.scalar.activation(
            sqscr[:, : fd - dve_f],
            xt[:, dve_f:],
            AF.Square,
            accum_out=qparts[:, T + i : T + i + 1],
        )
        # group drains: regions 0..5 leave PSUM on ACT while later tiles
        # still stream (2 ops), regions 6..7 go to DVE in the tail
        if resident and i == 5:
            for g in range(3):
                nc.scalar.activation(
                    drscr[:],
                    bigps[:C, g * 1024 : (g + 1) * 1024],
                    AF.Copy,
                    accum_out=st_s[:, g : g + 1],
                )
        elif not resident and (i % nreg == nreg - 1 or i == T - 1):
            lo = (i - i % nreg) % 1  # group start region is always 0
            nreg_used = i % nreg + 1
            nc.scalar.activation(
                drscr2[:, : nreg_used * 512],
                bigps[:C, : nreg_used * 512],
                AF.Copy,
                accum_out=st_s[:, i // nreg : i // nreg + 1],
            )

    if resident:
        # regions 6-7 drain on ACT, overlapping DVE's final stt
        nc.scalar.activation(
            drscr[:],
            bigps[:C, 3072:4096],
            AF.Copy,
            accum_out=st_s[:, 3:4],
        )
    ssum = const.tile([C, 1], dt.float32)
    nc.vector.tensor_reduce(out=ssum[:], in_=st_s[:], axis=AX.X, op=OP.add)
    qred = const.tile([P, 1], dt.float32)
    nc.vector.tensor_reduce(out=qred[:], in_=qparts[:], axis=AX.X, op=OP.add)
    # fold the RB row-blocks of each channel: [32, 1] = sel128.T @ qred
    nc.tensor.matmul(
        bigps[:C, 0:1], lhsT=sel128[:], rhs=qred[:], start=True, stop=True
    )

    mean = const.tile([C, 1], dt.float32)
    nc.vector.tensor_scalar_mul(mean[:], ssum[:], invn[:])
    ex2 = const.tile([C, 1], dt.float32)
    nc.vector.tensor_scalar_mul(ex2[:], bigps[:C, 0:1], invn[:])
    msq = const.tile([C, 1], dt.float32)
    nc.vector.tensor_mul(msq[:], mean[:], mean[:])
    var = const.tile([C, 1], dt.float32)
    nc.vector.tensor_sub(var[:], ex2[:], msq[:])
    std = const.tile([C, 1], dt.float32)
    nc.scalar.activation(std[:], var[:], AF.Sqrt, bias=epsv[:])
    istd = const.tile([C, 1], dt.float32)
    nc.vector.reciprocal(istd[:], std[:])
    # ab = [A | B]: A = w/std, B = b - mean*A
    ab = const.tile([C, 2], dt.float32)
    nc.vector.tensor_mul(ab[:, 0:1], istd[:], wt[:])
    nc.vector.tensor_mul(ab[:, 1:2], mean[:], ab[:, 0:1])
    nc.vector.tensor_sub(ab[:, 1:2], bt[:], ab[:, 1:2])
    # broadcast A/B back to all 128 partitions: [128, 2] = sel32.T @ ab
    nc.tensor.matmul(
        bigps[:, 512:514], lhsT=sel32[:], rhs=ab[:], start=True, stop=True
    )
    ab128 = const.tile([P, 2], dt.float32)
    nc.vector.tensor_copy(out=ab128[:], in_=bigps[:, 512:514])

    # Pass 2: smallest tile first (fast first store), then descending so
    # the final store is small too.
    if resident:
        by_size = sorted(range(T), key=lambda i: -fds[i])
        order = [by_size[-1]] + by_size[:-1]
    else:
        order = list(range(T))
    for n, i in enumerate(order):
        fd = fds[i]
        if i < RES:
            yt = xts[i]
        else:
            yt = ypool.tile([P, fd], dt.float16, tag="yt")
            nc.sync.dma_start(out=yt[:], in_=xv[:, offs[i] : offs[i] + fd])
        slot = (n % 3) * fdmax
        ot = obuf[:, slot : slot + fd]
        nc.vector.tensor_scalar(
            out=ot[:],
            in0=yt[:],
            scalar1=ab128[:, 0:1],
            scalar2=ab128[:, 1:2],
            op0=OP.mult,
            op1=OP.add,
        )
        # alternate stores across both HWDGE rings (sync is idle in pass 2)
        if n % 2 == 0:
            nc.scalar.dma_start(out=ov[:, offs[i] : offs[i] + fd], in_=ot[:])
        else:
            nc.sync.dma_start(out=ov[:, offs[i] : offs[i] + fd], in_=ot[:])


def _get_program(totf, fds, resident):
    key = (totf, tuple(fds), resident)
    if key in _PROGRAMS:
        return _PROGRAMS[key]
    import concourse.tile as tile
    from concourse import bacc, mybir

    dt = mybir.dt
    nc = bacc.Bacc(
        "TRN2",
        target_bir_lowering=False,
        debug=False,
        enable_asserts=False,
        num_devices=NCORES,
    )
    x_d = nc.dram_tensor("x", [P, totf], dt.float16, kind="ExternalInput")
    invn_d = nc.dram_tensor("invn", [C, 1], dt.float32, kind="ExternalInput")
    w_d = nc.dram_tensor("w", [C, 1], dt.float32, kind="ExternalInput")
    b_d = nc.dram_tensor("b", [C, 1], dt.float32, kind="ExternalInput")
    s128_d = nc.dram_tensor("sel128", [P, C], dt.float32, kind="ExternalInput")
    s16_d = nc.dram_tensor("sel16", [P, C], dt.float16, kind="ExternalInput")
    s32_d = nc.dram_tensor("sel32", [C, P], dt.float32, kind="ExternalInput")
    o_d = nc.dram_tensor("o", [P, totf], dt.float16, kind="ExternalOutput")

    tensors = (x_d, invn_d, w_d, b_d, s128_d, s16_d, s32_d, o_d)
    with tile.TileContext(nc) as tc:
        with ExitStack() as ctx:
            _emit(nc, tc, ctx, tensors, totf, fds, resident)

    nc.finalize()
    _PROGRAMS[key] = nc
    return nc


def _pack(rows, totf):
    """rows [n, C] f32 -> [128, TOTF] f16: partition rb*32+c, column f
    holds row rb*TOTF + f of channel c; zero padded."""
    xp = np.zeros((RB * totf, C), dtype=np.float16)
    xp[: rows.shape[0]] = rows
    return np.ascontiguousarray(
        xp.reshape(RB, totf, C).transpose(0, 2, 1).reshape(P, totf)
    )


def _unpack(slab, n):
    """[128, TOTF] f16 -> rows [n, C] f32."""
    totf = slab.shape[1]
    return (
        slab.reshape(RB, C, totf)
        .transpose(0, 2, 1)
        .reshape(RB * totf, C)[:n]
        .astype(np.float32)
    )


def kernel(feats, seg_ids, weight, bias, num_segments, **_):
    from concourse.bass_utils import run_bass_kernel_spmd

    feats = np.ascontiguousarray(np.asarray(feats), dtype=np.float32)
    seg = np.asarray(seg_ids)
    w = np.asarray(weight, dtype=np.float32).reshape(C, 1)
    b = np.asarray(bias, dtype=np.float32).reshape(C, 1)
    S = int(num_segments)
    N = feats.shape[0]

    assert (np.diff(seg) >= 0).all(), "seg_ids must be sorted"
    bounds = np.searchsorted(seg, np.arange(S + 1)).astype(np.int64)
    counts = np.diff(bounds)

    sel128 = np.ascontiguousarray(np.tile(np.eye(C, dtype=np.float32), (RB, 1)))
    sel32 = np.ascontiguousarray(sel128.T)
    sel16 = sel128.astype(np.float16)

    out = np.empty((N, C), dtype=np.float32)
    for g0 in range(0, S, NCORES):
        gsegs = list(range(g0, min(g0 + NCORES, S)))
        maxc = max(int(counts[s]) for s in gsegs)
        totf, fds, resident = _cfg(maxc)
        nc = _get_program(totf, fds, resident)
        in_maps = []
        for j in range(NCORES):
            n_j = 1
            if j < len(gsegs):
                s = gsegs[j]
                n_j = max(int(counts[s]), 1)
                rows = feats[bounds[s] : bounds[s + 1]]
            else:
                rows = np.zeros((0, C), dtype=np.float32)
            in_maps.append(
                {
                    "x": _pack(rows, totf),
                    "invn": np.full((C, 1), 1.0 / n_j, dtype=np.float32),
                    "w": w,
                    "b": b,
                    "sel128": sel128,
                    "sel16": sel16,
                    "sel32": sel32,
                }
            )
        r = run_bass_kernel_spmd(nc, in_maps, list(range(NCORES)))
        global LAST_RESULTS
        LAST_RESULTS = r
        results = r.results
        for j, s in enumerate(gsegs):
            out[bounds[s] : bounds[s + 1]] = _unpack(results[j]["o"], int(counts[s]))
    return out


# revision 27
# speedup vs baseline: 1.0922x; 1.0240x over previous
"""MinkowskiInstanceNorm (segment-reduce instance norm) on 8 Trainium2 cores.

Strategy: seg_ids are sorted, so each segment is a contiguous run of rows.
With num_segments == n_cores == 8, core j owns segment j outright: it
computes sum(x) and sum(x^2) over its rows (padded with zeros so one SPMD
program serves all cores), derives mean / inv_std / affine on-device, and
normalizes in a second pass.  No cross-core communication is needed; the
host only slices rows per segment and stitches the outputs back in order.

Perf design (HBM-bound problem; ~430 GB/s/core achievable):
  - The device slab is fp16 (host casts on pack, upcasts on unpack),
    halving HBM traffic vs fp32.  Quantization error ~5e-4 relative,
    far inside the correctness gate.
  - The whole per-core slab stays RESIDENT in SBUF across both passes,
    so HBM traffic is exactly one read + one write of the data.
  - Layout: channels on partitions — partition p = rb*32 + c (rb =
    row-block 0..3, c = channel), free axis = column f holding row
    rb*TOTF + f of that block; one flat [128, TOTF] slab, split into
    tiles of DESCENDING width so the last tile's compute tail after the
    final load is tiny.
  - Pass-1 per tile: PE folds sum(x) via 0/1-selector matmuls
    accumulated in a per-tile PSUM region (fp16 rhs streams ~3 cols/
    cycle, and PE is otherwise idle); sum(x^2) is split DVE
    (scalar_tensor_tensor, 1x ~1.07 ns/elem) / ACT (Square+accum,
    ~0.886 ns/elem); ACT also drains each tile's PSUM region as soon as
    that tile's matmuls finish, so nothing but the last tile's work
    lands in the stats gap.
  - Pass 2 normalizes smallest-tile-first with one 4x-mode DVE
    tensor_scalar per tile and alternates stores across both HWDGE
    rings (sync + ACT) for sustained store bandwidth.
"""

from contextlib import ExitStack

import numpy as np

C = 32  # channels
P = 128  # SBUF partitions
RB = P // C  # row blocks (4)
NCORES = 8
EPS = 1e-8
T_TILES = 8
# descending tile-width fractions (sum = 1); last tile ~6% so the
# compute tail after the final load is short
FRACS = [0.15, 0.15, 0.14, 0.14, 0.13, 0.12, 0.10, 0.07]
DVE_SHARE = 0.48  # leading share of each tile's sum(x^2) on DVE
# fully-resident limit: slab(2*TOTF) + obuf(2 slots) + scratch must fit
TOTF_RESIDENT_MAX = 65536
FD_STREAM = 8192  # uniform tile width for the streamed fallback

_PROGRAMS = {}
LAST_RESULTS = None  # BassKernelResults of the most recent SPMD launch


def _cfg(maxc):
    """Pick (totf, fds, resident) covering maxc rows (= RB*totf capacity)."""
    need = max(int(maxc), 1)
    totf = -(-need // (RB * 64)) * 64
    if totf <= TOTF_RESIDENT_MAX:
        if totf <= 8 * 64:
            return totf, [totf], True
        fds = []
        off = 0
        for i, fr in enumerate(FRACS):
            left = len(FRACS) - 1 - i
            if left == 0:
                fd = totf - off
            else:
                fd = max((int(totf * fr) // 64) * 64, 64)
                fd = min(fd, totf - off - 64 * left)
            if fd <= 0:
                break
            fds.append(fd)
            off += fd
        assert sum(fds) == totf and all(f > 0 for f in fds)
        return totf, fds, True
    t = -(-need // (RB * FD_STREAM))
    return t * FD_STREAM, [FD_STREAM] * t, False


def _emit(nc, tc, ctx, tensors, totf, fds, resident):
    from concourse import mybir

    dt = mybir.dt
    AX = mybir.AxisListType
    OP = mybir.AluOpType
    AF = mybir.ActivationFunctionType

    x_d, invn_d, w_d, b_d, s128_d, s16_d, s32_d, o_d = tensors
    xv = x_d.ap()  # [P, TOTF]
    ov = o_d.ap()
    T = len(fds)
    offs = [sum(fds[:i]) for i in range(T)]
    fdmax = max(fds)

    const = ctx.enter_context(tc.tile_pool(name="const", bufs=1))
    xpool = ctx.enter_context(tc.tile_pool(name="xpool", bufs=3))
    ypool = ctx.enter_context(tc.tile_pool(name="ypool", bufs=2))
    psum = ctx.enter_context(tc.tile_pool(name="psum", bufs=1, space="PSUM"))

    if resident:
        RES = T
        res = const.tile([P, totf], dt.float16, name="res")
    else:
        budget = 190 * 1024
        fixed = 2 * 2 * fdmax + 3 * fdmax + 2 * fdmax + 4096
        RES = min(T, max((budget - fixed - 10 * fdmax) // (fdmax * 2), 0))
        res = (
            const.tile([P, sum(fds[:RES])], dt.float16, name="res")
            if RES
            else None
        )

    # output ring buffer: 3 slots of the largest tile width
    obuf = const.tile([P, 3 * fdmax], dt.float16)
    dvemax = (int(fdmax * DVE_SHARE) // 32) * 32
    ttrscr = const.tile([P, dvemax], dt.float16)
    sqscr = const.tile([P, fdmax - dvemax + 32], dt.float16)
    drscr = const.tile([C, 1024], dt.float32)
    drscr2 = None if resident else const.tile([C, 4096], dt.float32)

    qparts = const.tile([P, 2 * T], dt.float32)
    st_s = const.tile([C, 8 if resident else -(-T // 8)], dt.float32)
    nc.vector.memset(st_s[:], 0.0)

    # Tile loads first so the sync-ring FIFO streams them back-to-back;
    # consts ride the gpsimd (SWDGE) ring so sel16 beats tile 0's matmul.
    for i in range(RES):
        nc.sync.dma_start(
            out=res[:, offs[i] : offs[i] + fds[i]],
            in_=xv[:, offs[i] : offs[i] + fds[i]],
        )

    sel16 = const.tile([P, C], dt.float16)
    nc.gpsimd.dma_start(out=sel16[:], in_=s16_d.ap())
    invn = const.tile([C, 1], dt.float32)
    nc.gpsimd.dma_start(out=invn[:], in_=invn_d.ap())
    wt = const.tile([C, 1], dt.float32)
    nc.gpsimd.dma_start(out=wt[:], in_=w_d.ap())
    bt = const.tile([C, 1], dt.float32)
    nc.gpsimd.dma_start(out=bt[:], in_=b_d.ap())
    sel128 = const.tile([P, C], dt.float32)
    nc.gpsimd.dma_start(out=sel128[:], in_=s128_d.ap())
    sel32 = const.tile([C, P], dt.float32)
    nc.gpsimd.dma_start(out=sel32[:], in_=s32_d.ap())

    # warm the ACT Sqrt table set now so the stats chain doesn't pay the
    # table load later
    epsv = const.tile([C, 1], dt.float32)
    nc.vector.memset(epsv[:], EPS)
    warm = const.tile([C, 1], dt.float32)
    nc.scalar.activation(warm[:], epsv[:], AF.Sqrt, bias=epsv[:])

    # one 512-col PSUM region per tile (slice r=i%8 of one big tile);
    # every chunk matmul writes cols [0:512) of its region (the final
    # partial chunk overlaps cols [0:rem)), so regions are fully valid
    # and group drains can span several regions at once.
    nreg = min(T, 8)
    bigps = psum.tile([P, nreg * 512], dt.float32)

    xts = []
    for i in range(T):
        if i < RES:
            xt = res[:, offs[i] : offs[i] + fds[i]]
        else:
            xt = xpool.tile([P, fds[i]], dt.float16, tag="xt")
            nc.sync.dma_start(out=xt[:], in_=xv[:, offs[i] : offs[i] + fds[i]])
        xts.append(xt)
        fd = fds[i]
        reg = bigps[:, (i % nreg) * 512 : (i % nreg + 1) * 512]
        # sum(x): chunk matmuls against the 0/1 selector, accumulated in
        # this tile's PSUM region (overlapping columns just add up)
        pos = 0
        first = True
        while pos < fd:
            w_cols = min(512, fd - pos)
            nc.tensor.matmul(
                reg[:C, :w_cols],
                lhsT=sel16[:],
                rhs=xt[:, pos : pos + w_cols],
                start=first,
                stop=(pos + w_cols >= fd),
            )
            first = False
            pos += w_cols
        # sum(x^2): leading DVE_F on DVE, trailing on ACT
        dve_f = (int(fd * DVE_SHARE) // 32) * 32
        nc.vector.scalar_tensor_tensor(
            out=ttrscr[:, :dve_f],
            in0=xt[:, :dve_f],
            scalar=1.0,
            in1=xt[:, :dve_f],
            op0=OP.mult,
            op1=OP.mult,
            accum_out=qparts[:, i : i + 1],
        )
        nc.scalar.activation(
            sqscr[:, : fd - dve_f],
            xt[:, dve_f:],
            AF.Square,
            accum_out=qparts[:, T + i : T + i + 1],
        )
        # group drains: regions 0..dA leave PSUM on ACT while later tiles
        # still stream; the remaining regions drain in the tail below
        if resident and i == min(5, T - 1):
            if all(f >= 512 for f in fds):
                cols = (i + 1) * 512
                for g, lo in enumerate(range(0, cols, 1024)):
                    nc.scalar.activation(
                        drscr[:, : min(1024, cols - lo)],
                        bigps[:C, lo : min(lo + 1024, cols)],
                        AF.Copy,
                        accum_out=st_s[:, g : g + 1],
                    )
            else:
                # narrow tiles leave stale PSUM beyond their width; drain
                # each region only as far as it was written (tiny inputs)
                for r in range(i + 1):
                    wr = min(512, fds[r])
                    nc.scalar.activation(
                        drscr[:, :wr],
                        bigps[:C, r * 512 : r * 512 + wr],
                        AF.Copy,
                        accum_out=st_s[:, r : r + 1],
                    )
        elif not resident and (i % nreg == nreg - 1 or i == T - 1):
            nreg_used = i % nreg + 1
            nc.scalar.activation(
                drscr2[:, : nreg_used * 512],
                bigps[:C, : nreg_used * 512],
                AF.Copy,
                accum_out=st_s[:, i // nreg : i // nreg + 1],
            )

    if resident and T > 6:
        # late regions drain on ACT, overlapping DVE's final stt
        if all(f >= 512 for f in fds):
            nc.scalar.activation(
                drscr[:, : (T - 6) * 512],
                bigps[:C, 3072 : T * 512],
                AF.Copy,
                accum_out=st_s[:, 3:4],
            )
        else:
            for r in range(6, T):
                wr = min(512, fds[r])
                nc.scalar.activation(
                    drscr[:, :wr],
                    bigps[:C, r * 512 : r * 512 + wr],
                    AF.Copy,
                    accum_out=st_s[:, r : r + 1],
                )
    ssum = const.tile([C, 1], dt.float32)
    nc.vector.tensor_reduce(out=ssum[:], in_=st_s[:], axis=AX.X, op=OP.add)
    qred = const.tile([P, 1], dt.float32)
    nc.vector.tensor_reduce(out=qred[:], in_=qparts[:], axis=AX.X, op=OP.add)
    # fold the RB row-blocks of each channel: [32, 1] = sel128.T @ qred
    nc.tensor.matmul(
        bigps[:C, 0:1], lhsT=sel128[:], rhs=qred[:], start=True, stop=True
    )

    mean = const.tile([C, 1], dt.float32)
    nc.vector.tensor_scalar_mul(mean[:], ssum[:], invn[:])
    ex2 = const.tile([C, 1], dt.float32)
    nc.vector.tensor_scalar_mul(ex2[:], bigps[:C, 0:1], invn[:])
    msq = const.tile([C, 1], dt.float32)
    nc.vector.tensor_mul(msq[:], mean[:], mean[:])
    var = const.tile([C, 1], dt.float32)
    nc.vector.tensor_sub(var[:], ex2[:], msq[:])
    std = const.tile([C, 1], dt.float32)
    nc.scalar.activation(std[:], var[:], AF.Sqrt, bias=epsv[:])
    istd = const.tile([C, 1], dt.float32)
    nc.vector.reciprocal(istd[:], std[:])
    # ab = [A | B]: A = w/std, B = b - mean*A
    ab = const.tile([C, 2], dt.float32)
    nc.vector.tensor_mul(ab[:, 0:1], istd[:], wt[:])
    nc.vector.tensor_mul(ab[:, 1:2], mean[:], ab[:, 0:1])
    nc.vector.tensor_sub(ab[:, 1:2], bt[:], ab[:, 1:2])
    # broadcast A/B back to all 128 partitions: [128, 2] = sel32.T @ ab
    nc.tensor.matmul(
        bigps[:, 512:514], lhsT=sel32[:], rhs=ab[:], start=True, stop=True
    )
    ab128 = const.tile([P, 2], dt.float32)
    nc.vector.tensor_copy(out=ab128[:], in_=bigps[:, 512:514])

    # Pass 2: smallest tile first (fast first store), then descending so
    # the final store is small too.
    if resident:
        by_size = sorted(range(T), key=lambda i: -fds[i])
        order = [by_size[-1]] + by_size[:-1]
    else:
        order = list(range(T))
    for n, i in enumerate(order):
        fd = fds[i]
        if i < RES:
            yt = xts[i]
        else:
            yt = ypool.tile([P, fd], dt.float16, tag="yt")
            nc.sync.dma_start(out=yt[:], in_=xv[:, offs[i] : offs[i] + fd])
        slot = (n % 3) * fdmax
        ot = obuf[:, slot : slot + fd]
        nc.vector.tensor_scalar(
            out=ot[:],
            in0=yt[:],
            scalar1=ab128[:, 0:1],
            scalar2=ab128[:, 1:2],
            op0=OP.mult,
            op1=OP.add,
        )
        # alternate stores across both HWDGE rings (sync is idle in pass 2)
        if n % 2 == 0:
            nc.scalar.dma_start(out=ov[:, offs[i] : offs[i] + fd], in_=ot[:])
        else:
            nc.sync.dma_start(out=ov[:, offs[i] : offs[i] + fd], in_=ot[:])


def _get_program(totf, fds, resident):
    key = (totf, tuple(fds), resident)
    if key in _PROGRAMS:
        return _PROGRAMS[key]
    import concourse.tile as tile
    from concourse import bacc, mybir

    dt = mybir.dt
    nc = bacc.Bacc(
        "TRN2",
        target_bir_lowering=False,
        debug=False,
        enable_asserts=False,
        num_devices=NCORES,
    )
    x_d = nc.dram_tensor("x", [P, totf], dt.float16, kind="ExternalInput")
    invn_d = nc.dram_tensor("invn", [C, 1], dt.float32, kind="ExternalInput")
    w_d = nc.dram_tensor("w", [C, 1], dt.float32, kind="ExternalInput")
    b_d = nc.dram_tensor("b", [C, 1], dt.float32, kind="ExternalInput")
    s128_d = nc.dram_tensor("sel128", [P, C], dt.float32, kind="ExternalInput")
    s16_d = nc.dram_tensor("sel16", [P, C], dt.float16, kind="ExternalInput")
    s32_d = nc.dram_tensor("sel32", [C, P], dt.float32, kind="ExternalInput")
    o_d = nc.dram_tensor("o", [P, totf], dt.float16, kind="ExternalOutput")

    tensors = (x_d, invn_d, w_d, b_d, s128_d, s16_d, s32_d, o_d)
    with tile.TileContext(nc) as tc:
        with ExitStack() as ctx:
            _emit(nc, tc, ctx, tensors, totf, fds, resident)

    nc.finalize()
    _PROGRAMS[key] = nc
    return nc


def _pack(rows, totf):
    """rows [n, C] f32 -> [128, TOTF] f16: partition rb*32+c, column f
    holds row rb*TOTF + f of channel c; zero padded."""
    xp = np.zeros((RB * totf, C), dtype=np.float16)
    xp[: rows.shape[0]] = rows
    return np.ascontiguousarray(
        xp.reshape(RB, totf, C).transpose(0, 2, 1).reshape(P, totf)
    )


def _unpack(slab, n):
    """[128, TOTF] f16 -> rows [n, C] f32."""
    totf = slab.shape[1]
    return (
        slab.reshape(RB, C, totf)
        .transpose(0, 2, 1)
        .reshape(RB * totf, C)[:n]
        .astype(np.float32)
    )


def kernel(feats, seg_ids, weight, bias, num_segments, **_):
    from concourse.bass_utils import run_bass_kernel_spmd

    feats = np.ascontiguousarray(np.asarray(feats), dtype=np.float32)
    seg = np.asarray(seg_ids)
    w = np.asarray(weight, dtype=np.float32).reshape(C, 1)
    b = np.asarray(bias, dtype=np.float32).reshape(C, 1)
    S = int(num_segments)
    N = feats.shape[0]

    assert (np.diff(seg) >= 0).all(), "seg_ids must be sorted"
    bounds = np.searchsorted(seg, np.arange(S + 1)).astype(np.int64)
    counts = np.diff(bounds)

    sel128 = np.ascontiguousarray(np.tile(np.eye(C, dtype=np.float32), (RB, 1)))
    sel32 = np.ascontiguousarray(sel128.T)
    sel16 = sel128.astype(np.float16)

    out = np.empty((N, C), dtype=np.float32)
    for g0 in range(0, S, NCORES):
        gsegs = list(range(g0, min(g0 + NCORES, S)))
        maxc = max(int(counts[s]) for s in gsegs)
        totf, fds, resident = _cfg(maxc)
        nc = _get_program(totf, fds, resident)
        in_maps = []
        for j in range(NCORES):
            n_j = 1
            if j < len(gsegs):
                s = gsegs[j]
                n_j = max(int(counts[s]), 1)
                rows = feats[bounds[s] : bounds[s + 1]]
            else:
                rows = np.zeros((0, C), dtype=np.float32)
            in_maps.append(
                {
                    "x": _pack(rows, totf),
                    "invn": np.full((C, 1), 1.0 / n_j, dtype=np.float32),
                    "w": w,
                    "b": b,
                    "sel128": sel128,
                    "sel16": sel16,
                    "sel32": sel32,
                }
            )
        r = run_bass_kernel_spmd(nc, in_maps, list(range(NCORES)))
        global LAST_RESULTS
        LAST_RESULTS = r
        results = r.results
        for j, s in enumerate(gsegs):
            out[bounds[s] : bounds[s + 1]] = _unpack(results[j]["o"], int(counts[s]))
    return out


# revision 29
# speedup vs baseline: 1.0927x; 1.0005x over previous
"""MinkowskiInstanceNorm (segment-reduce instance norm) on 8 Trainium2 cores.

Strategy: seg_ids are sorted, so each segment is a contiguous run of rows.
With num_segments == n_cores == 8, core j owns segment j outright: it
computes sum(x) and sum(x^2) over its rows (padded with zeros so one SPMD
program serves all cores), derives mean / inv_std / affine on-device, and
normalizes in a second pass.  No cross-core communication is needed; the
host only slices rows per segment and stitches the outputs back in order.

Perf design (HBM-bound problem; ~430 GB/s/core achievable):
  - The device slab is fp16 (host casts on pack, upcasts on unpack),
    halving HBM traffic vs fp32.  Quantization error ~5e-4 relative,
    far inside the correctness gate.
  - The whole per-core slab stays RESIDENT in SBUF across both passes,
    so HBM traffic is exactly one read + one write of the data.
  - Layout: channels on partitions — partition p = rb*32 + c (rb =
    row-block 0..3, c = channel), free axis = column f holding row
    rb*TOTF + f of that block; one flat [128, TOTF] slab, split into
    tiles of DESCENDING width so the last tile's compute tail after the
    final load is tiny.
  - Pass-1 per tile: PE folds sum(x) via 0/1-selector matmuls
    accumulated in a per-tile PSUM region (fp16 rhs streams ~3 cols/
    cycle, and PE is otherwise idle); sum(x^2) is split DVE
    (scalar_tensor_tensor, 1x ~1.07 ns/elem) / ACT (Square+accum,
    ~0.886 ns/elem); ACT drains the PSUM sum regions in grouped ops
    (regions 0-5 while late tiles still stream, the rest overlapping
    DVE's final stt), so only the last tile's work lands in the stats
    gap.  The ACT Sqrt table is warmed at startup.
  - Pass 2 normalizes smallest-tile-first with one 4x-mode DVE
    tensor_scalar per tile and alternates stores across both HWDGE
    rings (sync + ACT) for sustained store bandwidth.
"""

from contextlib import ExitStack

import numpy as np

C = 32  # channels
P = 128  # SBUF partitions
RB = P // C  # row blocks (4)
NCORES = 8
EPS = 1e-8
T_TILES = 8
# descending tile-width fractions (sum = 1); last tile ~6% so the
# compute tail after the final load is short
FRACS = [0.15, 0.15, 0.14, 0.14, 0.13, 0.12, 0.10, 0.07]
DVE_SHARE = 0.48  # leading share of each tile's sum(x^2) on DVE
# fully-resident limit: slab(2*TOTF) + obuf(2 slots) + scratch must fit
TOTF_RESIDENT_MAX = 65536
FD_STREAM = 8192  # uniform tile width for the streamed fallback

_PROGRAMS = {}
LAST_RESULTS = None  # BassKernelResults of the most recent SPMD launch


def _cfg(maxc):
    """Pick (totf, fds, resident) covering maxc rows (= RB*totf capacity)."""
    need = max(int(maxc), 1)
    totf = -(-need // (RB * 64)) * 64
    if totf <= TOTF_RESIDENT_MAX:
        if totf <= 8 * 64:
            return totf, [totf], True
        fds = []
        off = 0
        for i, fr in enumerate(FRACS):
            left = len(FRACS) - 1 - i
            if left == 0:
                fd = totf - off
            else:
                fd = max((int(totf * fr) // 64) * 64, 64)
                fd = min(fd, totf - off - 64 * left)
            if fd <= 0:
                break
            fds.append(fd)
            off += fd
        assert sum(fds) == totf and all(f > 0 for f in fds)
        return totf, fds, True
    t = -(-need // (RB * FD_STREAM))
    return t * FD_STREAM, [FD_STREAM] * t, False


def _emit(nc, tc, ctx, tensors, totf, fds, resident):
    from concourse import mybir

    dt = mybir.dt
    AX = mybir.AxisListType
    OP = mybir.AluOpType
    AF = mybir.ActivationFunctionType

    x_d, invn_d, w_d, b_d, s128_d, s16_d, s32_d, o_d = tensors
    xv = x_d.ap()  # [P, TOTF]
    ov = o_d.ap()
    T = len(fds)
    offs = [sum(fds[:i]) for i in range(T)]
    fdmax = max(fds)

    const = ctx.enter_context(tc.tile_pool(name="const", bufs=1))
    xpool = ctx.enter_context(tc.tile_pool(name="xpool", bufs=3))
    ypool = ctx.enter_context(tc.tile_pool(name="ypool", bufs=2))
    psum = ctx.enter_context(tc.tile_pool(name="psum", bufs=1, space="PSUM"))

    if resident:
        RES = T
        res = const.tile([P, totf], dt.float16, name="res")
    else:
        budget = 190 * 1024
        fixed = 2 * 2 * fdmax + 3 * fdmax + 2 * fdmax + 4096
        RES = min(T, max((budget - fixed - 10 * fdmax) // (fdmax * 2), 0))
        res = (
            const.tile([P, sum(fds[:RES])], dt.float16, name="res")
            if RES
            else None
        )

    # output ring buffer: 3 slots of the largest tile width
    obuf = const.tile([P, 3 * fdmax], dt.float16)
    dvemax = (int(fdmax * DVE_SHARE) // 32) * 32
    ttrscr = const.tile([P, dvemax], dt.float16)
    sqscr = const.tile([P, fdmax - dvemax + 32], dt.float16)
    drscr = const.tile([C, 1024], dt.float32)
    drscr2 = None if resident else const.tile([C, 4096], dt.float32)

    qparts = const.tile([P, 2 * T], dt.float32)
    st_s = const.tile([C, 8 if resident else -(-T // 8)], dt.float32)
    nc.vector.memset(st_s[:], 0.0)

    # Tile loads first so the sync-ring FIFO streams them back-to-back;
    # consts ride the gpsimd (SWDGE) ring so sel16 beats tile 0's matmul.
    for i in range(RES):
        nc.sync.dma_start(
            out=res[:, offs[i] : offs[i] + fds[i]],
            in_=xv[:, offs[i] : offs[i] + fds[i]],
        )

    sel16 = const.tile([P, C], dt.float16)
    nc.gpsimd.dma_start(out=sel16[:], in_=s16_d.ap())
    invn = const.tile([C, 1], dt.float32)
    nc.gpsimd.dma_start(out=invn[:], in_=invn_d.ap())
    wt = const.tile([C, 1], dt.float32)
    nc.gpsimd.dma_start(out=wt[:], in_=w_d.ap())
    bt = const.tile([C, 1], dt.float32)
    nc.gpsimd.dma_start(out=bt[:], in_=b_d.ap())
    sel128 = const.tile([P, C], dt.float32)
    nc.gpsimd.dma_start(out=sel128[:], in_=s128_d.ap())
    sel32 = const.tile([C, P], dt.float32)
    nc.gpsimd.dma_start(out=sel32[:], in_=s32_d.ap())

    # warm the ACT Sqrt table set now so the stats chain doesn't pay the
    # table load later
    epsv = const.tile([C, 1], dt.float32)
    nc.vector.memset(epsv[:], EPS)
    warm = const.tile([C, 1], dt.float32)
    nc.scalar.activation(warm[:], epsv[:], AF.Sqrt, bias=epsv[:])

    # one 512-col PSUM region per tile (slice r=i%8 of one big tile);
    # every chunk matmul writes cols [0:512) of its region (the final
    # partial chunk overlaps cols [0:rem)), so regions are fully valid
    # and group drains can span several regions at once.
    nreg = min(T, 8)
    bigps = psum.tile([P, max(nreg, 2) * 512], dt.float32)

    xts = []
    for i in range(T):
        if i < RES:
            xt = res[:, offs[i] : offs[i] + fds[i]]
        else:
            xt = xpool.tile([P, fds[i]], dt.float16, tag="xt")
            nc.sync.dma_start(out=xt[:], in_=xv[:, offs[i] : offs[i] + fds[i]])
        xts.append(xt)
        fd = fds[i]
        reg = bigps[:, (i % nreg) * 512 : (i % nreg + 1) * 512]
        # sum(x): chunk matmuls against the 0/1 selector, accumulated in
        # this tile's PSUM region (overlapping columns just add up)
        pos = 0
        first = True
        while pos < fd:
            w_cols = min(512, fd - pos)
            nc.tensor.matmul(
                reg[:C, :w_cols],
                lhsT=sel16[:],
                rhs=xt[:, pos : pos + w_cols],
                start=first,
                stop=(pos + w_cols >= fd),
            )
            first = False
            pos += w_cols
        # sum(x^2): leading DVE_F on DVE, trailing on ACT
        dve_f = (int(fd * DVE_SHARE) // 32) * 32
        nc.vector.scalar_tensor_tensor(
            out=ttrscr[:, :dve_f],
            in0=xt[:, :dve_f],
            scalar=1.0,
            in1=xt[:, :dve_f],
            op0=OP.mult,
            op1=OP.mult,
            accum_out=qparts[:, i : i + 1],
        )
        nc.scalar.activation(
            sqscr[:, : fd - dve_f],
            xt[:, dve_f:],
            AF.Square,
            accum_out=qparts[:, T + i : T + i + 1],
        )
        # group drains: regions 0..dA leave PSUM on ACT while later tiles
        # still stream; the remaining regions drain in the tail below
        if resident and i == min(5, T - 1):
            if all(f >= 512 for f in fds):
                cols = (i + 1) * 512
                for g, lo in enumerate(range(0, cols, 1024)):
                    nc.scalar.activation(
                        drscr[:, : min(1024, cols - lo)],
                        bigps[:C, lo : min(lo + 1024, cols)],
                        AF.Copy,
                        accum_out=st_s[:, g : g + 1],
                    )
            else:
                # narrow tiles leave stale PSUM beyond their width; drain
                # each region only as far as it was written (tiny inputs)
                for r in range(i + 1):
                    wr = min(512, fds[r])
                    nc.scalar.activation(
                        drscr[:, :wr],
                        bigps[:C, r * 512 : r * 512 + wr],
                        AF.Copy,
                        accum_out=st_s[:, r : r + 1],
                    )
        elif not resident and (i % nreg == nreg - 1 or i == T - 1):
            nreg_used = i % nreg + 1
            nc.scalar.activation(
                drscr2[:, : nreg_used * 512],
                bigps[:C, : nreg_used * 512],
                AF.Copy,
                accum_out=st_s[:, i // nreg : i // nreg + 1],
            )

    if resident and T > 6:
        # late regions drain on ACT, overlapping DVE's final stt
        if all(f >= 512 for f in fds):
            nc.scalar.activation(
                drscr[:, : (T - 6) * 512],
                bigps[:C, 3072 : T * 512],
                AF.Copy,
                accum_out=st_s[:, 3:4],
            )
        else:
            for r in range(6, T):
                wr = min(512, fds[r])
                nc.scalar.activation(
                    drscr[:, :wr],
                    bigps[:C, r * 512 : r * 512 + wr],
                    AF.Copy,
                    accum_out=st_s[:, r : r + 1],
                )
    ssum = const.tile([C, 1], dt.float32)
    nc.vector.tensor_reduce(out=ssum[:], in_=st_s[:], axis=AX.X, op=OP.add)
    qred = const.tile([P, 1], dt.float32)
    nc.vector.tensor_reduce(out=qred[:], in_=qparts[:], axis=AX.X, op=OP.add)
    # fold the RB row-blocks of each channel: [32, 1] = sel128.T @ qred
    nc.tensor.matmul(
        bigps[:C, 0:1], lhsT=sel128[:], rhs=qred[:], start=True, stop=True
    )

    mean = const.tile([C, 1], dt.float32)
    nc.vector.tensor_scalar_mul(mean[:], ssum[:], invn[:])
    ex2 = const.tile([C, 1], dt.float32)
    nc.vector.tensor_scalar_mul(ex2[:], bigps[:C, 0:1], invn[:])
    msq = const.tile([C, 1], dt.float32)
    nc.vector.tensor_mul(msq[:], mean[:], mean[:])
    var = const.tile([C, 1], dt.float32)
    nc.vector.tensor_sub(var[:], ex2[:], msq[:])
    std = const.tile([C, 1], dt.float32)
    nc.scalar.activation(std[:], var[:], AF.Sqrt, bias=epsv[:])
    istd = const.tile([C, 1], dt.float32)
    nc.vector.reciprocal(istd[:], std[:])
    # ab = [A | B]: A = w/std, B = b - mean*A
    ab = const.tile([C, 2], dt.float32)
    nc.vector.tensor_mul(ab[:, 0:1], istd[:], wt[:])
    nc.vector.tensor_mul(ab[:, 1:2], mean[:], ab[:, 0:1])
    nc.vector.tensor_sub(ab[:, 1:2], bt[:], ab[:, 1:2])
    # broadcast A/B back to all 128 partitions: [128, 2] = sel32.T @ ab
    nc.tensor.matmul(
        bigps[:, 512:514], lhsT=sel32[:], rhs=ab[:], start=True, stop=True
    )
    ab128 = const.tile([P, 2], dt.float32)
    nc.vector.tensor_copy(out=ab128[:], in_=bigps[:, 512:514])

    # Pass 2: smallest tile first (fast first store), then descending so
    # the final store is small too.
    if resident:
        by_size = sorted(range(T), key=lambda i: -fds[i])
        order = [by_size[-1]] + by_size[:-1]
    else:
        order = list(range(T))
    for n, i in enumerate(order):
        fd = fds[i]
        if i < RES:
            yt = xts[i]
        else:
            yt = ypool.tile([P, fd], dt.float16, tag="yt")
            nc.sync.dma_start(out=yt[:], in_=xv[:, offs[i] : offs[i] + fd])
        slot = (n % 3) * fdmax
        ot = obuf[:, slot : slot + fd]
        nc.vector.tensor_scalar(
            out=ot[:],
            in0=yt[:],
            scalar1=ab128[:, 0:1],
            scalar2=ab128[:, 1:2],
            op0=OP.mult,
            op1=OP.add,
        )
        # alternate stores across both HWDGE rings (sync is idle in pass 2)
        if n % 2 == 0:
            nc.scalar.dma_start(out=ov[:, offs[i] : offs[i] + fd], in_=ot[:])
        else:
            nc.sync.dma_start(out=ov[:, offs[i] : offs[i] + fd], in_=ot[:])


def _get_program(totf, fds, resident):
    key = (totf, tuple(fds), resident)
    if key in _PROGRAMS:
        return _PROGRAMS[key]
    import concourse.tile as tile
    from concourse import bacc, mybir

    dt = mybir.dt
    nc = bacc.Bacc(
        "TRN2",
        target_bir_lowering=False,
        debug=False,
        enable_asserts=False,
        num_devices=NCORES,
    )
    x_d = nc.dram_tensor("x", [P, totf], dt.float16, kind="ExternalInput")
    invn_d = nc.dram_tensor("invn", [C, 1], dt.float32, kind="ExternalInput")
    w_d = nc.dram_tensor("w", [C, 1], dt.float32, kind="ExternalInput")
    b_d = nc.dram_tensor("b", [C, 1], dt.float32, kind="ExternalInput")
    s128_d = nc.dram_tensor("sel128", [P, C], dt.float32, kind="ExternalInput")
    s16_d = nc.dram_tensor("sel16", [P, C], dt.float16, kind="ExternalInput")
    s32_d = nc.dram_tensor("sel32", [C, P], dt.float32, kind="ExternalInput")
    o_d = nc.dram_tensor("o", [P, totf], dt.float16, kind="ExternalOutput")

    tensors = (x_d, invn_d, w_d, b_d, s128_d, s16_d, s32_d, o_d)
    with tile.TileContext(nc) as tc:
        with ExitStack() as ctx:
            _emit(nc, tc, ctx, tensors, totf, fds, resident)

    nc.finalize()
    _PROGRAMS[key] = nc
    return nc


def _pack(rows, totf):
    """rows [n, C] f32 -> [128, TOTF] f16: partition rb*32+c, column f
    holds row rb*TOTF + f of channel c; zero padded."""
    xp = np.zeros((RB * totf, C), dtype=np.float16)
    xp[: rows.shape[0]] = rows
    return np.ascontiguousarray(
        xp.reshape(RB, totf, C).transpose(0, 2, 1).reshape(P, totf)
    )


def _unpack(slab, n):
    """[128, TOTF] f16 -> rows [n, C] f32."""
    totf = slab.shape[1]
    return (
        slab.reshape(RB, C, totf)
        .transpose(0, 2, 1)
        .reshape(RB * totf, C)[:n]
        .astype(np.float32)
    )


def kernel(feats, seg_ids, weight, bias, num_segments, **_):
    from concourse.bass_utils import run_bass_kernel_spmd

    feats = np.ascontiguousarray(np.asarray(feats), dtype=np.float32)
    seg = np.asarray(seg_ids)
    w = np.asarray(weight, dtype=np.float32).reshape(C, 1)
    b = np.asarray(bias, dtype=np.float32).reshape(C, 1)
    S = int(num_segments)
    N = feats.shape[0]

    assert (np.diff(seg) >= 0).all(), "seg_ids must be sorted"
    bounds = np.searchsorted(seg, np.arange(S + 1)).astype(np.int64)
    counts = np.diff(bounds)

    sel128 = np.ascontiguousarray(np.tile(np.eye(C, dtype=np.float32), (RB, 1)))
    sel32 = np.ascontiguousarray(sel128.T)
    sel16 = sel128.astype(np.float16)

    out = np.empty((N, C), dtype=np.float32)
    for g0 in range(0, S, NCORES):
        gsegs = list(range(g0, min(g0 + NCORES, S)))
        maxc = max(int(counts[s]) for s in gsegs)
        totf, fds, resident = _cfg(maxc)
        nc = _get_program(totf, fds, resident)
        in_maps = []
        for j in range(NCORES):
            n_j = 1
            if j < len(gsegs):
                s = gsegs[j]
                n_j = max(int(counts[s]), 1)
                rows = feats[bounds[s] : bounds[s + 1]]
            else:
                rows = np.zeros((0, C), dtype=np.float32)
            in_maps.append(
                {
                    "x": _pack(rows, totf),
                    "invn": np.full((C, 1), 1.0 / n_j, dtype=np.float32),
                    "w": w,
                    "b": b,
                    "sel128": sel128,
                    "sel16": sel16,
                    "sel32": sel32,
                }
            )
        r = run_bass_kernel_spmd(nc, in_maps, list(range(NCORES)))
        global LAST_RESULTS
        LAST_RESULTS = r
        results = r.results
        for j, s in enumerate(gsegs):
            out[bounds[s] : bounds[s + 1]] = _unpack(results[j]["o"], int(counts[s]))
    return out
